# revision 2
# baseline (speedup 1.0000x reference)
"""CondConv2d (MoE routed conv) Trainium2 Bass kernel — bf16 + 1D Winograd.

Strategy
--------
Data-parallel over batch B=32 across 8 NeuronCores (4 samples/core).
Per core and sample, the 3x3 conv is computed with Winograd F(2,3) along
W (1.5x fewer PE MACs) and direct 3-tap convolution along H:

  1. Host prep: x is padded, cast to bf16, and laid out as the four
     F(2,3)-transformed planes X_nu[c, hp, tw] (a fixed linear
     re-layout, like the padding/transposes); experts are
     G-transformed along kw, relayout'ed to [C, E, 3*4, O] bf16.
  2. Prologue: tiny X1-plane DMAs at the head of the DMA queue feed
     pooled sums, so routing for samples 0/1 (and pooled for 2/3) all
     resolve in the first ~10us; routing for b+2 re-runs mid-loop where
     its pooled is already resident, so the in-order PE queue never
     waits on it.  (Timeline-sim showed the old 1-sample-ahead routing
     chain head-of-line blocking PE/DVE/ACT for ~10us per sample.)
  3. Expert mixing one sample ahead, all on DVE: 4 tensor_scalar_mul
     (4x perf mode, ~260ns/768 cols — the old scalar_tensor_tensor
     chain ran at 1x, 860ns) + a 3-add TT tree at 2x.
  4. Conv: for each output chunk [128o] and 14-row block, accumulate
     2(cc) x 3(di) bf16 matmuls of N=392 into 4 nu-planes in PSUM,
     consuming cmb slices in mixing-group order (subtile deps let the
     first MMs start before the whole cmb is mixed).
  5. Evacuation: ACT copies the nu-planes to SBUF (PSUM banks free
     fast); DVE computes the packed halves of the F(2,3) inverse
     (te=M0+M1, to=M1-M2 at 2x) and GPSIMD absorbs the strided
     interleaving ops (even=te+M2, odd=to-M3, 1x-rate anywhere);
     one 6272B/partition HWDGE DMA per ochunk to HBM (splitting any
     of the big DMAs was descriptor-overhead bound on HW).
  6. The For_i timing loop passes hint_engines for all five engines:
     the body far exceeds one 16KiB IRAM block per engine, and unhinted
     back-edge I$ misses cost ~70us/iteration of pure measurement
     artifact.

Engine busy per sample (timeline-sim): PE ~33us (the bottleneck:
192 N=392 bf16 MMs), DVE ~27 (mixing 19 + inverse 4 + pooled), ACT ~17
(evac copies), Pool ~4 + strided inverse, DMA ~18 (in 3.3MB + out
1.6MB + slab share).  Loop-slope on HW: ~137-145us/iteration
(baseline: 233us measured the same way without branch hints: ~208us
from the kernel changes alone).
"""

import os
import numpy as np
import ml_dtypes
from contextlib import ExitStack

import concourse.bass as bass
import concourse.bacc as bacc
import concourse.mybir as mybir
import concourse.tile as tile
from concourse.bass_utils import run_bass_kernel_spmd

F32 = mybir.dt.float32
BF16 = mybir.dt.bfloat16
AF = mybir.ActivationFunctionType
ALU = mybir.AluOpType
AX = mybir.AxisListType

# Problem shapes (hardcoded per contract).
B, C, H, W = 32, 256, 56, 56
E, O, K = 4, 256, 3
HID = 64
NCORES = 8
BL = B // NCORES          # samples per core
CCH = C // 128            # c partition chunks
OCH = O // 128            # o partition chunks
HP, WP = H + 2, W + 2     # padded
DI = 3                    # H taps (direct)
NU = 4                    # Winograd F(2,3) positions along W
TW = W // 2               # 28 Winograd tiles per row
RBW = 14                  # output rows per block
NBW = H // RBW            # 4 row blocks
NW = RBW * TW             # 392 = matmul free size
KSEG = DI * NU            # 12 transformed taps
SEG = KSEG * O            # 3072 = comb cols per c-chunk
XCOLS = NU * HP * TW      # 6496 = transformed-x cols per c-chunk

_CACHE = {}


def _build_program(reps=1, loop_n=None):
    nc = bacc.Bacc("TRN2", target_bir_lowering=False, debug=False)

    x_d = nc.dram_tensor("xt", [BL, C, XCOLS], BF16, kind="ExternalInput").ap()
    ex_d = nc.dram_tensor("experts_t", [C, E, KSEG, O], BF16,
                          kind="ExternalInput").ap()
    # packed routing params: [:,0:64]=rw1t cc0, [:,64:128]=rw1t cc1,
    # [0:64,128]=rb1, [0:64,129:133]=rw2t, [0:1,133:137]=rb2
    RP = 2 * HID + 1 + E + E
    rp_d = nc.dram_tensor("rparams", [128, RP], F32, kind="ExternalInput").ap()
    # output in bf16 (host upcasts): the kernel is DMA-bandwidth bound
    # (~110-140 GB/s/core effective), so halving output bytes wins
    out_d = nc.dram_tensor("out", [BL, O, H, W], BF16,
                           kind="ExternalOutput").ap()

    with tile.TileContext(nc) as tc, ExitStack() as ctx:
        const_pool = ctx.enter_context(tc.tile_pool(name="const", bufs=1))
        xstg_pool = ctx.enter_context(tc.tile_pool(name="xstg", bufs=4))
        xw_pool = ctx.enter_context(tc.tile_pool(name="xw", bufs=3 * CCH))
        comb_pool = ctx.enter_context(tc.tile_pool(name="comb", bufs=2 * CCH))
        scr_pool = ctx.enter_context(tc.tile_pool(name="scr", bufs=8))
        etmp_pool = ctx.enter_context(tc.tile_pool(name="etmp", bufs=12))
        ostg_pool = ctx.enter_context(tc.tile_pool(name="ostg", bufs=2))
        small_pool = ctx.enter_context(tc.tile_pool(name="small", bufs=2))
        cpsum_pool = ctx.enter_context(tc.tile_pool(name="cps", bufs=8,
                                                    space="PSUM"))

        # ---- constants / parameters (preload once, single DMA) ----
        rp_t = const_pool.tile([128, RP], F32, name="rp")
        nc.sync.dma_start(rp_t[:], rp_d[:])
        rw1t_t = [rp_t[:, 0:HID], rp_t[:, HID:2 * HID]]
        rb1_t = rp_t[0:HID, 2 * HID:2 * HID + 1]
        rw2t_t = rp_t[0:HID, 2 * HID + 1:2 * HID + 1 + E]
        rb2_t = rp_t[0:1, 2 * HID + 1 + E:2 * HID + 1 + 2 * E]
        ones_t = const_pool.tile([1, 128], F32, name="ones")
        nc.vector.memset(ones_t[:], 1.0)

        NG = 4                    # mixing groups (of 3 taps each)
        GSZ = SEG // NG           # 768
        slabs = []   # [cc][e][g] -> [128, GSZ] tile

        def emit_slab_alloc():
            slabs.clear()
            for cc in range(CCH):
                slabs.append([None] * E)
                for e in range(E):
                    slabs[cc][e] = const_pool.tile([128, SEG], BF16,
                                                   name=f"slab{cc}e{e}")

        def emit_slab_loads(ccs=None):
            exv = ex_d.rearrange("c e k o -> c e (k o)")
            if os.environ.get("SLABV", "0") == "0":
                # one DMA per (cc, e): 6KB/partition contiguous descriptors
                for cc in (ccs or range(CCH)):
                    for e in range(E):
                        nc.sync.dma_start(
                            slabs[cc][e][:],
                            exv[cc * 128:(cc + 1) * 128, e, :])
                return
            # group-streamed: 4 chunk-DMAs per (cc, e), ordered (cc, g, e) so
            # mixing group (cc, g) can start after 4 small DMAs instead of
            # the whole 6.3MB bank (prologue fill)
            for cc in (ccs or range(CCH)):
                for g in range(NG):
                    lo = g * GSZ
                    for e in range(E):
                        nc.sync.dma_start(
                            slabs[cc][e][:, lo:lo + GSZ],
                            exv[cc * 128:(cc + 1) * 128, e, lo:lo + GSZ])

        # per-sample state
        xw = {}       # (b, cc) -> [128, NU, HP, TW] bf16 transformed input
        comb = {}     # (b, cc) -> combined weights [128, SEG] bf16
        pooled_t = {}  # (b, cc) -> [128,1] h*w sums

        px_t = {}

        def emit_px_dma(b):
            # small dedicated DMAs of just the X1 plane so pooled (and thus
            # routing) for EVERY sample can run in the prologue, off the
            # critical PE/DVE order of the steady-state loop
            for cc in range(CCH):
                px = scr_pool.tile([128, HP * TW], BF16, tag="px", bufs=4,
                                   name=f"px{b}_{cc}")
                nc.sync.dma_start(px[:], x_d[b, cc * 128:(cc + 1) * 128,
                                              HP * TW:2 * HP * TW])
                px_t[(b, cc)] = px

        xw_raw = {}

        def emit_x1_first(b):
            # for samples whose xw tile already exists in the prologue, the
            # early X1-plane DMA goes straight into the xw tile (no px dup)
            pt = HP * TW
            for cc in range(CCH):
                t = xw_pool.tile([128, XCOLS], BF16, tag="xw",
                                 name=f"xw{b}_{cc}")
                nc.sync.dma_start(t[:, pt:2 * pt],
                                  x_d[b, cc * 128:(cc + 1) * 128, pt:2 * pt])
                xw_raw[(b, cc)] = t
                xw[(b, cc)] = t.rearrange("p (nu h tw) -> p nu h tw",
                                          nu=NU, tw=TW)
                px_t[(b, cc)] = t[:, pt:2 * pt]

        def emit_loads_rest(b):
            # remaining planes of an xw tile created by emit_x1_first
            pt = HP * TW
            for cc in range(CCH):
                t = xw_raw[(b, cc)]
                nc.sync.dma_start(t[:, 0:pt],
                                  x_d[b, cc * 128:(cc + 1) * 128, 0:pt])
                nc.sync.dma_start(t[:, 2 * pt:XCOLS],
                                  x_d[b, cc * 128:(cc + 1) * 128,
                                      2 * pt:XCOLS])

        def emit_pooled_reduce(b, on_act=False):
            for cc in range(CCH):
                p = small_pool.tile([128, 1], F32, tag="pooled", bufs=8,
                                    name=f"pool{b}_{cc}")
                if on_act:
                    trash = scr_pool.tile([128, HP * TW], BF16, tag="ptrash",
                                          bufs=2, name=f"ptr{b}_{cc}")
                    nc.scalar.activation(trash[:], px_t[(b, cc)][:],
                                         AF.Copy, accum_out=p[:])
                else:
                    nc.vector.reduce_sum(out=p[:], in_=px_t[(b, cc)][:],
                                         axis=AX.X)
                pooled_t[(b, cc)] = [p]

        def emit_pooled_early(b):
            emit_px_dma(b)
            emit_pooled_reduce(b)

        def emit_loads(b, do_pooled=True, ccs=None):
            # X planes are host-transformed (the input DMA path hides fully
            # under the PE; the serialized OUTPUT path is what matters);
            # reduce the X1 plane for the routing pool (its elements tile
            # each unpadded row exactly once).
            for cc in (ccs if ccs is not None else range(CCH)):
                t = xw_pool.tile([128, XCOLS], BF16, tag="xw",
                                 name=f"xw{b}_{cc}")
                if os.environ.get("XLV", "0") == "1":
                    # X1 plane first: pooled (and thus routing) unblocks
                    # after 0.8MB instead of 1.7MB
                    pt = HP * TW
                    nc.sync.dma_start(t[:, pt:2 * pt],
                                      x_d[b, cc * 128:(cc + 1) * 128,
                                          pt:2 * pt])
                    nc.sync.dma_start(t[:, 0:pt],
                                      x_d[b, cc * 128:(cc + 1) * 128, 0:pt])
                    nc.sync.dma_start(t[:, 2 * pt:XCOLS],
                                      x_d[b, cc * 128:(cc + 1) * 128,
                                          2 * pt:XCOLS])
                else:
                    half = XCOLS // 2
                    nc.sync.dma_start(t[:, 0:half],
                                      x_d[b, cc * 128:(cc + 1) * 128, 0:half])
                    nc.sync.dma_start(t[:, half:XCOLS],
                                      x_d[b, cc * 128:(cc + 1) * 128,
                                          half:XCOLS])
                if not do_pooled:
                    xw[(b, cc)] = t.rearrange("p (nu h tw) -> p nu h tw",
                                              nu=NU, tw=TW)
                    continue
                p = small_pool.tile([128, 1], F32, tag="pooled", bufs=6,
                                    name=f"pool{b}_{cc}")
                plv = os.environ.get("PLV", "0")
                if plv == "1":
                    # row-sum on ACT via activation(Copy, accum_out=...) into
                    # a scratch tile: frees DVE of the 1x-rate reduce
                    trash = scr_pool.tile([128, HP * TW], BF16, tag="ptrash",
                                          bufs=2, name=f"ptr{b}_{cc}")
                    nc.scalar.activation(trash[:], t[:, HP * TW:2 * HP * TW],
                                         AF.Copy, accum_out=p[:])
                elif plv == "2":
                    # row-sum rides a GPSIMD copy (accum_out): Pool is idle
                    # early, so pooled lands fast and never head-of-line
                    # blocks the evacuation path on ACT/DVE
                    trash = scr_pool.tile([128, HP * TW], BF16, tag="ptrash",
                                          bufs=2, name=f"ptr{b}_{cc}")
                    nc.gpsimd.tensor_scalar(trash[:],
                                            t[:, HP * TW:2 * HP * TW],
                                            1.0, None, op0=ALU.mult,
                                            accum_out=p[:])
                elif plv == "3":
                    with tc.high_priority():
                        nc.vector.reduce_sum(out=p[:],
                                             in_=t[:, HP * TW:2 * HP * TW],
                                             axis=AX.X)
                else:
                    nc.vector.reduce_sum(out=p[:],
                                         in_=t[:, HP * TW:2 * HP * TW],
                                         axis=AX.X)
                xw[(b, cc)] = t.rearrange("p (nu h tw) -> p nu h tw",
                                          nu=NU, tw=TW)
                pooled_t[(b, cc)] = [p]

        def emit_routing(b):
            if os.environ.get("PRIOV", "0") == "1":
                with tc.high_priority():
                    return emit_routing_body(b)
            return emit_routing_body(b)

        def emit_routing_body(b):
            mps = cpsum_pool.tile([128, NW], F32, tag="cps", name=f"mps{b}")
            parts = [(cc, p) for cc in range(CCH)
                     for p in pooled_t[(b, cc)]]
            for i, (cc, p) in enumerate(parts):
                nc.tensor.matmul(mps[0:HID, 0:1], rw1t_t[cc], p[:],
                                 start=(i == 0), stop=(i == len(parts) - 1))
            h_sb = small_pool.tile([HID, 1], F32, tag="h", name=f"h{b}")
            nc.scalar.activation(h_sb[:], mps[0:HID, 0:1], AF.Relu, bias=rb1_t[:])
            nc.tensor.matmul(mps[0:1, 4:4 + E], h_sb[:], rw2t_t[:],
                             start=True, stop=True)
            ze = small_pool.tile([1, E], F32, tag="ze", name=f"ze{b}")
            nc.vector.tensor_add(ze[:], mps[0:1, 4:4 + E], rb2_t[:])
            es = small_pool.tile([1, E], F32, tag="es", name=f"es{b}")
            nc.scalar.activation(es[:], ze[:], AF.Exp)
            ssum = small_pool.tile([1, 1], F32, tag="ssum", name=f"ss{b}")
            nc.vector.reduce_sum(out=ssum[:], in_=es[:], axis=AX.X)
            rec = small_pool.tile([1, 1], F32, tag="rec", name=f"rec{b}")
            nc.vector.reciprocal(rec[:], ssum[:])
            esn = small_pool.tile([1, E], F32, tag="esn", name=f"esn{b}")
            nc.vector.tensor_scalar_mul(esn[:], es[:], rec[:, 0:1])
            nc.tensor.matmul(mps[0:128, 8:8 + E], ones_t[:], esn[:],
                             start=True, stop=True)
            rbc = small_pool.tile([128, E], F32, tag="rbc", name=f"rbc{b}")
            nc.scalar.copy(rbc[:], mps[0:128, 8:8 + E])
            return rbc

        MIXV = os.environ.get("MIXV", "1")

        def emit_mixing(b, rbc, fast=False):
            if os.environ.get("PRIOV", "0") == "1":
                with tc.high_priority():
                    return emit_mixing_body(b, rbc, fast)
            return emit_mixing_body(b, rbc, fast)

        def emit_mixing_body(b, rbc, fast=False):
            # v2 (MIXV>=1): 4 tensor_scalar_mul on DVE (4x perf mode, ~260ns
            # per 768 cols vs 860ns for the 1x STT chain), then a tree of
            # plain TT adds (2x). MIXV=2 puts the u2+u3 add on GPSIMD.
            if MIXV == "0":
                return emit_mixing_v0(b, rbc, fast)
            for cc in range(CCH):
                slab = slabs[cc]
                cmb = comb_pool.tile([128, SEG], BF16, tag="comb",
                                     name=f"cmb{b}_{cc}")
                for g in range(NG):
                    lo = g * GSZ
                    us = []
                    for e in range(E):
                        u = scr_pool.tile([128, GSZ], BF16, tag="scr",
                                          name=f"u{b}_{cc}_{g}_{e}")
                        nc.vector.tensor_scalar_mul(
                            u[:], slab[e][:, lo:lo + GSZ], rbc[:, e:e + 1])
                        us.append(u)
                    t01 = scr_pool.tile([128, GSZ], BF16, tag="scr2", bufs=4,
                                        name=f"t01_{b}_{cc}_{g}")
                    nc.vector.tensor_add(t01[:], us[0][:], us[1][:])
                    t23 = scr_pool.tile([128, GSZ], BF16, tag="scr2", bufs=4,
                                        name=f"t23_{b}_{cc}_{g}")
                    if MIXV == "2":
                        nc.gpsimd.tensor_add(t23[:], us[2][:], us[3][:])
                    else:
                        nc.vector.tensor_add(t23[:], us[2][:], us[3][:])
                    nc.vector.tensor_add(cmb[:, lo:lo + GSZ], t01[:], t23[:])
                comb[(b, cc)] = cmb

        def emit_mixing_v0(b, rbc, fast=False):
            # chain heads (r_0 * slab) on ACT, accumulate STT steps on DVE.
            # fast=True (pipeline prologue): odd groups instead use 4 ACT
            # scale-muls + a GPSIMD add-tree, halving the DVE serial latency
            # before the first conv can start.
            for cc in range(CCH):
                slab = slabs[cc]
                cmb = comb_pool.tile([128, SEG], BF16, tag="comb",
                                     name=f"cmb{b}_{cc}")
                for g in range(NG):
                    lo = g * GSZ
                    if fast and g % 2 == 1:
                        us = []
                        for e in range(E):
                            u = scr_pool.tile([128, GSZ], BF16, tag="scr",
                                              name=f"u{b}_{cc}_{g}_{e}")
                            nc.scalar.mul(u[:], slab[e][:, lo:lo + GSZ],
                                          rbc[:, e:e + 1])
                            us.append(u)
                        t01 = scr_pool.tile([128, GSZ], BF16, tag="scr",
                                            name=f"t01_{b}_{cc}_{g}")
                        nc.gpsimd.tensor_add(t01[:], us[0][:], us[1][:])
                        t23 = scr_pool.tile([128, GSZ], BF16, tag="scr",
                                            name=f"t23_{b}_{cc}_{g}")
                        nc.gpsimd.tensor_add(t23[:], us[2][:], us[3][:])
                        nc.gpsimd.tensor_add(cmb[:, lo:lo + GSZ],
                                             t01[:], t23[:])
                        continue
                    a = scr_pool.tile([128, GSZ], BF16, tag="scr",
                                      name=f"scr{b}_{cc}_{g}")
                    if fast:
                        nc.vector.tensor_scalar_mul(
                            a[:], slab[0][:, lo:lo + GSZ], rbc[:, 0:1])
                    else:
                        nc.scalar.mul(a[:], slab[0][:, lo:lo + GSZ], rbc[:, 0:1])
                    for e in range(1, E - 1):
                        nc.vector.scalar_tensor_tensor(
                            a[:], slab[e][:, lo:lo + GSZ],
                            rbc[:, e:e + 1], a[:], op0=ALU.mult, op1=ALU.add)
                    nc.vector.scalar_tensor_tensor(
                        cmb[:, lo:lo + GSZ],
                        slab[E - 1][:, lo:lo + GSZ],
                        rbc[:, E - 1:E], a[:], op0=ALU.mult, op1=ALU.add)
                comb[(b, cc)] = cmb

        def emit_conv_ochunk(b, oc):
            # one staging tile + ONE output DMA per ochunk: 6272B/partition
            # contiguous descriptors (per-block DMAs at 1568B/desc were
            # descriptor-overhead bound on the serial DMA path)
            st = ostg_pool.tile([128, H * W], BF16, tag="ostg",
                                name=f"st{b}_{oc}")
            stf = st.rearrange("p (h tw two) -> p h tw two", two=2, tw=TW)
            for blk in range(NBW):
                ptiles = [cpsum_pool.tile([128, NW], F32, tag="cps",
                                          name=f"cp{b}_{oc}_{blk}_{nu}")
                          for nu in range(NU)]
                # consume cmb slices in mixing-group completion order
                # (cc, g, kidx) so each MM can start as soon as its group is
                # mixed (subtile deps) instead of after the whole cmb
                occ = [0] * NU
                for cc in range(CCH):
                    cmb = comb[(b, cc)]
                    xwv = xw[(b, cc)]
                    for kidx in range(KSEG):
                        di, nu = divmod(kidx, NU)
                        first = occ[nu] == 0
                        last = occ[nu] == 2 * DI - 1
                        occ[nu] += 1
                        w_ap = cmb[:, kidx * O + oc * 128:
                                   kidx * O + oc * 128 + 128]
                        rhs = xwv[:, nu,
                                  blk * RBW + di: blk * RBW + di + RBW, :]
                        nc.tensor.matmul(ptiles[nu][:], w_ap, rhs,
                                         start=first, stop=last)
                if os.environ.get("WSTRIP") == "noevac":
                    continue
                stv = stf[:, blk * RBW:(blk + 1) * RBW]
                INVV = os.environ.get("INVV", "3")
                if INVV == "1":
                    # hybrid: ACT evacuates only M1/M2 (the reused planes);
                    # DVE reads M0/M3 straight from PSUM.
                    cp1 = etmp_pool.tile([128, NW], BF16, tag="etmp",
                                         name=f"c1{b}_{oc}_{blk}")
                    nc.scalar.copy(cp1[:], ptiles[1][:])
                    cp2 = etmp_pool.tile([128, NW], BF16, tag="etmp",
                                         name=f"c2{b}_{oc}_{blk}")
                    nc.scalar.copy(cp2[:], ptiles[2][:])
                    c1v = cp1.rearrange("p (h tw) -> p h tw", tw=TW)
                    c2v = cp2.rearrange("p (h tw) -> p h tw", tw=TW)
                    p0v = ptiles[0].rearrange("p (h tw) -> p h tw", tw=TW)
                    p3v = ptiles[3].rearrange("p (h tw) -> p h tw", tw=TW)
                    te = etmp_pool.tile([128, NW], BF16, tag="etmp",
                                        name=f"te{b}_{oc}_{blk}")
                    to = etmp_pool.tile([128, NW], BF16, tag="etmp",
                                        name=f"to{b}_{oc}_{blk}")
                    tev = te.rearrange("p (h tw) -> p h tw", tw=TW)
                    tov = to.rearrange("p (h tw) -> p h tw", tw=TW)
                    nc.vector.tensor_add(tev[:], p0v[:], c1v[:])
                    nc.vector.tensor_add(stv[:, :, :, 0], tev[:], c2v[:])
                    nc.vector.tensor_sub(tov[:], c1v[:], c2v[:])
                    nc.vector.tensor_sub(stv[:, :, :, 1], tov[:], p3v[:])
                    continue
                # ACT copies free the PSUM banks fast (converting to bf16);
                # DVE does the F(2,3) inverse on the SBUF copies
                # (even = (M0+M1)+M2, odd = (M1-M2)-M3)
                cp = []
                for nu in range(NU):
                    c = etmp_pool.tile([128, NW], BF16, tag="etmp",
                                       name=f"c{b}_{oc}_{blk}_{nu}")
                    nc.scalar.copy(c[:], ptiles[nu][:])
                    cp.append(c.rearrange("p (h tw) -> p h tw", tw=TW))
                te = etmp_pool.tile([128, NW], BF16, tag="etmp",
                                    name=f"te{b}_{oc}_{blk}")
                to = etmp_pool.tile([128, NW], BF16, tag="etmp",
                                    name=f"to{b}_{oc}_{blk}")
                tev = te.rearrange("p (h tw) -> p h tw", tw=TW)
                tov = to.rearrange("p (h tw) -> p h tw", tw=TW)
                nc.vector.tensor_add(tev[:], cp[0][:], cp[1][:])
                nc.vector.tensor_sub(tov[:], cp[1][:], cp[2][:])
                if INVV == "3":
                    # the strided-output (interleaving) ops run at 1x on DVE
                    # anyway; GPSIMD absorbs them (inverse is slack-tolerant:
                    # PSUM is released by the ACT copies above)
                    nc.gpsimd.tensor_add(stv[:, :, :, 0], tev[:], cp[2][:])
                    nc.gpsimd.tensor_sub(stv[:, :, :, 1], tov[:], cp[3][:])
                else:
                    nc.vector.tensor_add(stv[:, :, :, 0], tev[:], cp[2][:])
                    nc.vector.tensor_sub(stv[:, :, :, 1], tov[:], cp[3][:])
            if os.environ.get("WSTRIP") not in ("noout", "noevac"):
                # issue output DMAs from the (otherwise idle) Pool queue and
                # the DVE queue (which runs the inverse this DMA depends on,
                # so no foreign dependencies), overlapping the SP input
                # stream AND each other (all-Pool measured 222.7us vs 235.1
                # on SP; ACT variants were worse — its copies gate on PE)
                eng = nc.gpsimd if oc == 0 else nc.sync
                if os.environ.get("OUTV", "0") == "1":
                    # per-2-block stores (3136B/partition) so the tail of
                    # each ochunk's store overlaps its last blocks' inverse
                    hh = RBW * 2 * W
                    for half in range(2):
                        eng.dma_start(
                            out_d[b, oc * 128:(oc + 1) * 128,
                                  half * 2 * RBW:(half + 1) * 2 * RBW, :],
                            st[:, half * hh:(half + 1) * hh])
                else:
                    eng.dma_start(
                        out_d[b, oc * 128:(oc + 1) * 128, :, :], st[:])

        # ---- emission: software-pipelined across samples ----
        # x DMA runs TWO samples ahead; routing + mixing ONE sample ahead,
        # so the tiny routing matmuls at the head of the in-order PE queue
        # never wait and comb[b+1] has a full conv window to mix.
        def emit_pipeline():
            if os.environ.get("RT2", "4") == "5":
                # ALL FOUR routing chains in the prologue (their px DMAs are
                # at the head of the DMA queue, ~9us), so the steady loop is
                # purely mixing+conv+evac: no routing op ever blocks the
                # in-order PE/DVE/ACT queues mid-stream.
                pxact = os.environ.get("PXACT", "0") == "1"
                emit_slab_alloc()
                emit_x1_first(0)
                emit_x1_first(1)
                emit_px_dma(2)
                emit_px_dma(3)
                route = {}
                emit_pooled_reduce(0)
                route[0] = emit_routing(0)
                emit_pooled_reduce(1)
                route[1] = emit_routing(1)
                emit_pooled_reduce(2, on_act=pxact)
                route[2] = emit_routing(2)
                emit_pooled_reduce(3, on_act=pxact)
                route[3] = emit_routing(3)
                emit_slab_loads([0])
                emit_loads_rest(0)
                emit_slab_loads([1])
                emit_mixing(0, route[0])
                emit_loads_rest(1)
                for b in range(BL):
                    if b + 1 < BL:
                        emit_mixing(b + 1, route[b + 1])
                    emit_conv_ochunk(b, 0)
                    if b + 2 < BL:
                        emit_loads(b + 2, do_pooled=False)
                    emit_conv_ochunk(b, 1)
                return
            if os.environ.get("RT2", "4") in ("4", "6", "7"):
                # pooled from tiny px DMAs (ALL queued FIRST: 3.2MB lands by
                # ~9us, so no engine ever head-of-line waits on them);
                # routing(0,1) in the prologue, routing(b+2) mid-loop where
                # its pooled is already resident.
                pxact = os.environ.get("PXACT", "0") == "1"
                emit_slab_alloc()
                emit_x1_first(0)
                emit_x1_first(1)
                emit_px_dma(2)
                emit_px_dma(3)
                emit_pooled_reduce(0)
                route = {}
                route[0] = emit_routing(0)
                emit_pooled_reduce(1)
                route[1] = emit_routing(1)
                emit_slab_loads([0])
                emit_loads_rest(0)
                emit_slab_loads([1])
                emit_mixing(0, route[0])
                emit_loads_rest(1)
                emit_pooled_reduce(2, on_act=pxact)
                emit_pooled_reduce(3, on_act=pxact)
                for b in range(BL):
                    if b + 2 < BL and os.environ.get("RT2", "4") == "7":
                        # routing(b+2) at the head of the iteration: pooled
                        # is prologue-resident, so the tiny chain drains at
                        # the sample boundary without blocking any conv MM
                        route[b + 2] = emit_routing(b + 2)
                    if b + 1 < BL:
                        emit_mixing(b + 1, route[b + 1])
                    emit_conv_ochunk(b, 0)
                    if b + 2 < BL:
                        emit_loads(b + 2, do_pooled=False)
                        if os.environ.get("RT2", "4") == "4":
                            route[b + 2] = emit_routing(b + 2)
                    emit_conv_ochunk(b, 1)
                    if b + 2 < BL and os.environ.get("RT2", "4") == "6":
                        route[b + 2] = emit_routing(b + 2)
                return
            if os.environ.get("RT2", "4") == "3":
                # ALL routing in the prologue: tiny X1-plane DMAs feed
                # pooled+routing for every sample up front, so the steady
                # loop's PE/DVE/ACT order contains only mixing+conv+evac.
                # DMA queue order is prologue-critical: px0, slab-cc0,
                # x0-cc0, slab-cc1, x0-cc1 minimizes the bytes before
                # conv(0) can stream at full rate.
                emit_slab_alloc()
                emit_pooled_early(0)
                route = {}
                route[0] = emit_routing(0)
                emit_slab_loads([0])
                emit_loads(0, do_pooled=False, ccs=[0])
                emit_slab_loads([1])
                emit_loads(0, do_pooled=False, ccs=[1])
                emit_pooled_early(1)
                route[1] = emit_routing(1)
                emit_mixing(0, route[0])
                emit_loads(1, do_pooled=False)
                emit_pooled_early(2)
                emit_pooled_early(3)
                route[2] = emit_routing(2)
                route[3] = emit_routing(3)
                for b in range(BL):
                    if b + 1 < BL:
                        emit_mixing(b + 1, route[b + 1])
                    emit_conv_ochunk(b, 0)
                    if b + 2 < BL:
                        emit_loads(b + 2, do_pooled=False)
                    emit_conv_ochunk(b, 1)
                return
            if os.environ.get("RT2", "4") == "1":
                # routing runs TWO samples ahead so rbc is ready the moment
                # mixing(b+1) is scheduled; mixing then wins the DVE heap
                # (high_priority) over the slack-tolerant inverse ops
                emit_loads(0)
                emit_slab_alloc()
                emit_slab_loads()
                route = {0: emit_routing(0)}
                emit_mixing(0, route[0])
                if BL > 1:
                    emit_loads(1)
                    route[1] = emit_routing(1)
                for b in range(BL):
                    if b + 1 < BL:
                        emit_mixing(b + 1, route[b + 1])
                    emit_conv_ochunk(b, 0)
                    if b + 2 < BL:
                        emit_loads(b + 2)
                        route[b + 2] = emit_routing(b + 2)
                    emit_conv_ochunk(b, 1)
                return
            emit_loads(0)
            emit_slab_alloc()
            emit_slab_loads()
            route = {0: emit_routing(0)}
            emit_mixing(0, route[0], fast=True)
            if BL > 1:
                emit_loads(1)
            for b in range(BL):
                if b + 1 < BL:
                    route[b + 1] = emit_routing(b + 1)
                    emit_mixing(b + 1, route[b + 1])
                emit_conv_ochunk(b, 0)
                if b + 2 < BL:
                    emit_loads(b + 2)
                emit_conv_ochunk(b, 1)

        if loop_n is not None:
            ET = mybir.EngineType
            hints = (() if os.environ.get("HINTE", "1") == "0" else
                     (ET.PE, ET.DVE, ET.Activation, ET.Pool, ET.SP))
            with tc.For_i(0, loop_n, 1, hint_engines=hints):
                for _rep in range(reps):
                    emit_pipeline()
        else:
            for _rep in range(reps):
                emit_pipeline()

    nc.compile()
    return nc


def _prep_inputs(x, experts, rw1, rb1, rw2, rb2):
    x = np.asarray(x, dtype=np.float32)
    xp = np.pad(x, ((0, 0), (0, 0), (1, 1), (1, 1)))
    ev = xp[..., 0::2]
    od = xp[..., 1::2]
    xt = np.stack([ev[..., 0:TW] - ev[..., 1:TW + 1],
                   od[..., 0:TW] + ev[..., 1:TW + 1],
                   ev[..., 1:TW + 1] - od[..., 0:TW],
                   od[..., 0:TW] - od[..., 1:TW + 1]],
                  axis=2)
    xt = np.ascontiguousarray(xt.reshape(B, C, XCOLS)).astype(
        ml_dtypes.bfloat16)
    experts = np.asarray(experts, dtype=np.float32)
    # F(2,3) weight transform along kw: [E,O,C,ki,kj] -> [C,E,ki*4,O]
    g0 = experts[..., 0]
    g1 = experts[..., 1]
    g2 = experts[..., 2]
    wt = np.stack([g0, (g0 + g1 + g2) * 0.5, (g0 - g1 + g2) * 0.5, g2],
                  axis=-1)                               # [E,O,C,ki,4]
    ex_t = np.ascontiguousarray(
        np.transpose(wt, (2, 0, 3, 4, 1)).reshape(C, E, KSEG, O)
    ).astype(ml_dtypes.bfloat16)
    rw1t = (np.asarray(rw1, dtype=np.float32) / float(H * W)).T  # [C, HID]
    rb1v = np.asarray(rb1, dtype=np.float32)
    rw2t = np.asarray(rw2, dtype=np.float32).T                   # [HID, E]
    rb2v = np.asarray(rb2, dtype=np.float32)
    RP = 2 * HID + 1 + 2 * E
    rp = np.zeros((128, RP), np.float32)
    rp[:, 0:HID] = rw1t[0:128]
    rp[:, HID:2 * HID] = rw1t[128:256]
    rp[0:HID, 2 * HID] = rb1v
    rp[0:HID, 2 * HID + 1:2 * HID + 1 + E] = rw2t
    rp[0, 2 * HID + 1 + E:2 * HID + 1 + 2 * E] = rb2v
    in_maps = []
    for i in range(NCORES):
        in_maps.append({
            "xt": np.ascontiguousarray(xt[i * BL:(i + 1) * BL]),
            "experts_t": ex_t,
            "rparams": rp,
        })
    return in_maps


def run(inputs, trace=False, **trace_kwargs):
    """Build (cached), run on 8 cores, return (full_out, BassKernelResults)."""
    key = "prog"
    if key not in _CACHE:
        _CACHE[key] = _build_program()
    nc = _CACHE[key]
    in_maps = _prep_inputs(**inputs)
    res = run_bass_kernel_spmd(nc, in_maps, list(range(NCORES)),
                               trace=trace, **trace_kwargs)
    out = np.concatenate([res.results[i]["out"] for i in range(NCORES)],
                         axis=0).astype(np.float32)
    return out, res


def kernel(x, experts, rw1, rb1, rw2, rb2):
    out, _ = run(dict(x=x, experts=experts, rw1=rw1, rb1=rb1, rw2=rw2, rb2=rb2))
    return out

